# revision 14
# baseline (speedup 1.0000x reference)
"""Trainium2 Bass kernel for nn_DeformableBlock (offset-conv -> deformable
conv v1 -> GroupNorm(32) -> ReLU), 8-core SPMD.

Sharding: core c -> (batch b = c//2, row-half h = c%2), rows [32h, 32h+32).
GroupNorm statistics are AllReduce'd across each (b,0)/(b,1) core pair.

v3 design:
  - z_k = x . W_k per 3x3 tap over a 40-row window, bf16 matmuls (fp32 psum),
    two taps per matmul (512-col moving operand).
  - z stored to DRAM in a doubled-row layout zd[q] = [z[q], z[q+64]] (bf16),
    so ONE gathered element (elem_size=1024, elem_step=512) fetches all 4
    bilinear corners: (y0,x0),(y1,x0),(y0,x1),(y1,x1). 18 dma_gathers total
    (9 taps x 2 halves of 1024 idx) -- SWDGE descriptor gen is the scarce
    resource. Slot weights are equality-adjusted for the y/x edge clamps.
  - gather index layout ([16-partition wrap, replicated x8]) built ON-CHIP via
    8 selection matmuls (partition shuffle), no DRAM bounce.
  - z stores round-robin across sync/scalar engine DMA queues.
"""
import functools
import numpy as np
import ml_dtypes

import concourse.bass as bass
import concourse.bacc as bacc
import concourse.mybir as mybir
import concourse.tile as tile
from concourse.bass_utils import run_bass_kernel_spmd

F32 = mybir.dt.float32
BF16 = mybir.dt.bfloat16
F16 = mybir.dt.float16
I16 = mybir.dt.int16
I32 = mybir.dt.int32
AOP = mybir.AluOpType
ACT = mybir.ActivationFunctionType

B, CIN, COUT, H, W = 4, 256, 256, 64, 64
K = 9
WROWS = 40            # z window rows (rows r0-4 .. r0+35)
XROWS = 35            # padded x slice rows (offset conv only; +1 slack row)
XCOLS = 66
NPOS = 2048           # output positions per core (32 rows)
NWIN = WROWS * 64     # z window positions (2560)
NT = 16               # output position tiles of 128
WT = 20               # window position tiles of 128
HT = WT // 2          # tiles per half-window store
ZDR = NWIN + 2        # zd rows (incl pad)
EPS = 1e-5
GN_N = 2 * NPOS * 8   # elements per GN group (both cores of the pair)

bf16 = ml_dtypes.bfloat16


def build_program(reps=1, use_cc=True):
    nc = bacc.Bacc(None, target_bir_lowering=False, num_devices=8)

    # ---------------- I/O ----------------
    xsl_d = nc.dram_tensor("xsl", [2, 128, XROWS, XCOLS], F16, kind="ExternalInput")
    xz_d = nc.dram_tensor("xz", [2, 128, NWIN], F16, kind="ExternalInput")
    wdef_d = nc.dram_tensor("wdef", [2, 128, K, COUT], F16, kind="ExternalInput")
    woff_d = nc.dram_tensor("woff", [2, 128, K, 18], F16, kind="ExternalInput")
    byc_d = nc.dram_tensor("byc", [128, NT, K], F32, kind="ExternalInput")
    bxc_d = nc.dram_tensor("bxc", [128, NT, K], F32, kind="ExternalInput")
    # per-core scalars replicated to [128,*]: idx offset, window y clamp lo/hi
    wconst_d = nc.dram_tensor("wconst", [128, 4], F32, kind="ExternalInput")
    # partition-shuffle matrices: pmat[s, u, p] = 1 iff s == 16u + p%16
    pmat_d = nc.dram_tensor("pmat", [128, 8, 128], F32, kind="ExternalInput")
    ident_d = nc.dram_tensor("ident", [128, 128], F32, kind="ExternalInput")
    onescol_d = nc.dram_tensor("onescol", [128, 1], F32, kind="ExternalInput")
    onesrow_d = nc.dram_tensor("onesrow", [1, 128], F32, kind="ExternalInput")
    gnab_d = nc.dram_tensor("gnab", [1, 512], F32, kind="ExternalInput")
    out_d = nc.dram_tensor("out", [NPOS, COUT], F32, kind="ExternalOutput")

    with tile.TileContext(nc) as tc:
        with (
            tc.tile_pool(name="const", bufs=1) as cpool,
            tc.tile_pool(name="wm", bufs=1) as wmpool,
            tc.tile_pool(name="zst", bufs=3) as zstpool,
            tc.tile_pool(name="g", bufs=2) as gpool,
            tc.tile_pool(name="acc", bufs=1) as accpool,
            tc.tile_pool(name="outp", bufs=2) as outpool,
            tc.tile_pool(name="ps", bufs=3, space="PSUM") as pspool,
            tc.tile_pool(name="ps2", bufs=1, space="PSUM") as ps2pool,
            tc.tile_pool(name="dram", bufs=1, space="DRAM") as dpool,
        ):
            # ---------------- load constants / inputs ----------------
            xsl = cpool.tile([128, 2, XROWS, XCOLS], F16, tag="xsl", name="xsl")
            for ci in range(2):
                nc.sync.dma_start(xsl[:, ci], xsl_d[ci])
            xz = cpool.tile([128, 2, NWIN], F16, tag="xz", name="xz")
            for ci in range(2):
                nc.sync.dma_start(xz[:, ci], xz_d[ci])
            wdef = cpool.tile([128, 2, K, COUT], F16, tag="wdef", name="wdef")
            woff = cpool.tile([128, 2, K, 18], F16, tag="woff", name="woff")
            for ci in range(2):
                nc.sync.dma_start(wdef[:, ci], wdef_d[ci])
                nc.sync.dma_start(woff[:, ci], woff_d[ci])
            byc = cpool.tile([128, NT, K], F32, tag="byc", name="byc")
            bxc = cpool.tile([128, NT, K], F32, tag="bxc", name="bxc")
            nc.sync.dma_start(byc[:], byc_d[:])
            nc.sync.dma_start(bxc[:], bxc_d[:])
            wconst = cpool.tile([128, 4], F32, tag="wconst", name="wconst")
            nc.sync.dma_start(wconst[:], wconst_d[:])
            pmat = cpool.tile([128, 8, 128], F32, tag="pmat", name="pmat")
            nc.sync.dma_start(pmat[:], pmat_d[:])
            ident = cpool.tile([128, 128], F32, tag="ident", name="ident")
            nc.sync.dma_start(ident[:], ident_d[:])
            onescol = cpool.tile([128, 1], F32, tag="onescol", name="onescol")
            nc.sync.dma_start(onescol[:], onescol_d[:])
            onesrow = cpool.tile([1, 128], F32, tag="onesrow", name="onesrow")
            nc.sync.dma_start(onesrow[:], onesrow_d[:])
            gnab = cpool.tile([1, 512], F32, tag="gnab", name="gnab")
            nc.sync.dma_start(gnab[:], gnab_d[:])

            # one doubled-row dram tile per tap: zd[q] = [z[q], z[q+64]]
            zds = [dpool.tile([ZDR, 2 * COUT], F16, tag=f"zd{k}",
                              name=f"zd{k}") for k in range(K)]
            ccin = dpool.tile([1, 64], F32, tag="ccin", name="ccin")
            ccout = dpool.tile([1, 64], F32, tag="ccout", name="ccout")

            # z matmuls for a pair of taps (or single for the last), plus the
            # doubled-layout stores. eng alternates the issuing DMA queue.
            def z_tap_group(kp, npair):
                fw = 512 if npair == 2 else 256
                for hw in range(2):
                    zst = zstpool.tile([128, HT, 512], F16, tag="zst", name="zst")
                    for tt in range(HT):
                        t = HT * hw + tt
                        zps = pspool.tile([128, 512], F32, tag="zps", name="zps")
                        nc.tensor.matmul(
                            zps[:, 0:fw], xz[:, 0, 128 * t:128 * (t + 1)],
                            wdef[:, 0, kp:kp + npair, :]
                            .rearrange("p a b -> p (a b)"),
                            start=True, stop=False)
                        nc.tensor.matmul(
                            zps[:, 0:fw], xz[:, 1, 128 * t:128 * (t + 1)],
                            wdef[:, 1, kp:kp + npair, :]
                            .rearrange("p a b -> p (a b)"),
                            start=False, stop=True)
                        nc.scalar.copy(zst[:, tt, 0:fw], zps[:, 0:fw])
                    for j in range(npair):
                        k = kp + j
                        eng = [nc.sync, nc.scalar, nc.gpsimd][(2 * k + hw) % 3]
                        zb = zds[k][:]
                        src = zst[:, :, 256 * j:256 * (j + 1)]
                        # write1: zd[q][0:256] = z[q],  q = hw*1280 + 128t + p
                        wr = bass.AP(zb.tensor, zb.offset + hw * 1280 * 512,
                                     [[512, 128], [128 * 512, HT], [1, 256]])
                        eng.dma_start(wr, src)
                        # write2: zd[q-64][256:512] = z[q]
                        if hw == 0:
                            wr = bass.AP(zb.tensor, zb.offset + 256,
                                         [[512, 64], [1, 256]])
                            eng.dma_start(wr, zst[64:128, 0, 256 * j:256 * (j + 1)])
                            wr = bass.AP(zb.tensor, zb.offset + 64 * 512 + 256,
                                         [[512, 128], [128 * 512, HT - 1], [1, 256]])
                            eng.dma_start(wr, zst[:, 1:HT, 256 * j:256 * (j + 1)])
                        else:
                            wr = bass.AP(zb.tensor,
                                         zb.offset + (1280 - 64) * 512 + 256,
                                         [[512, 128], [128 * 512, HT], [1, 256]])
                            eng.dma_start(wr, src)

            for _rep in range(reps):
                # ---------------- offset conv: [18, 2048] via im2col ----------
                # moving operand streams full padded rows (66 cols incl junk),
                # junk skipped at evacuation
                off_sb = cpool.tile([18, NPOS], F32, tag="off_sb", name="off_sb")
                xsl_flat = xsl[:].rearrange("p c r x -> p c (r x)")
                for q in range(6):  # 6-row chunks of output rows (last is 2)
                    nrows = 6 if q < 5 else 2
                    span = nrows * XCOLS
                    ops = ps2pool.tile([18, 6 * XCOLS], F32, tag="offps", name="offps")
                    first = True
                    for k in range(K):
                        ky, kx = k // 3, k % 3
                        base = (6 * q + ky) * XCOLS + kx
                        nc.tensor.matmul(
                            ops[:, 0:span], woff[:, 0, k, :],
                            xsl_flat[:, 0, base:base + span],
                            start=first, stop=False)
                        first = False
                        nc.tensor.matmul(
                            ops[:, 0:span], woff[:, 1, k, :],
                            xsl_flat[:, 1, base:base + span],
                            start=False, stop=(k == K - 1))
                    nc.scalar.copy(
                        off_sb[:, 384 * q:384 * q + 64 * nrows]
                        .rearrange("p (r x) -> p r x", x=64),
                        ops[:, 0:span].rearrange("p (r x) -> p r x", x=XCOLS)[:, :, 0:64])

                # PE-transpose offsets to position-major [128, NT, 18]
                offt = cpool.tile([128, NT, 18], F32, tag="offt", name="offt")
                for t in range(NT):
                    tps = ps2pool.tile([128, 18], F32, tag="tps", name="tps")
                    nc.tensor.transpose(
                        tps[:], off_sb[:, 128 * t:128 * (t + 1)], ident[0:18, 0:18])
                    nc.vector.tensor_copy(offt[:, t, :], tps[:])

                # ---------------- bilinear weights + indices (fp32, DVE) ------
                def wm(tag):
                    return wmpool.tile([128, NT, K], F32, tag=tag, name=tag)

                py = wm("py"); px = wm("px")
                # lifted sample coords: byc/bxc carry +16 and the offset bias
                nc.vector.tensor_add(py[:], offt[:, :, 0:18:2], byc[:])
                nc.vector.tensor_add(px[:], offt[:, :, 1:18:2], bxc[:])

                def dev_floor(src, tag):
                    ii = wmpool.tile([128, NT, K], I32, tag=tag + "i", name=tag + "i")
                    ff = wm(tag + "f")
                    gt = wm(tag + "g")
                    nc.vector.tensor_copy(ii[:], src[:])        # fp32 -> int32
                    nc.vector.tensor_copy(ff[:], ii[:])         # int32 -> fp32
                    nc.vector.tensor_tensor(gt[:], ff[:], src[:], op=AOP.is_gt)
                    nc.vector.tensor_tensor(ff[:], ff[:], gt[:], op=AOP.subtract)
                    return ff

                y0 = dev_floor(py, "y0")
                x0 = dev_floor(px, "x0")
                ty = wm("ty"); tx = wm("tx")
                nc.vector.tensor_tensor(ty[:], py[:], y0[:], op=AOP.subtract)
                nc.vector.tensor_tensor(tx[:], px[:], x0[:], op=AOP.subtract)
                y1 = wm("y1"); x1 = wm("x1")
                nc.vector.tensor_scalar_add(y1[:], y0[:], 1.0)
                nc.vector.tensor_scalar_add(x1[:], x0[:], 1.0)

                # global validity (lifted bounds [16, 79])
                def valid(src, tag):
                    g = wm(tag + "c")
                    v = wm(tag + "v")
                    nc.vector.tensor_scalar(g[:], src[:], 16.0, 79.0,
                                            op0=AOP.max, op1=AOP.min)
                    nc.vector.tensor_tensor(v[:], g[:], src[:], op=AOP.is_equal)
                    return v

                vy0 = valid(y0, "vy0"); vy1 = valid(y1, "vy1")
                vx0 = valid(x0, "vx0"); vx1 = valid(x1, "vx1")

                # gather pair bases: y row to [wlo, whi-1], x col to [16, 78]
                gy = wm("gy"); gx = wm("gx")
                nc.vector.tensor_scalar(gy[:], y0[:], wconst[:, 1:2],
                                        wconst[:, 2:3], op0=AOP.max, op1=AOP.min)
                nc.vector.tensor_scalar(gx[:], x0[:], 16.0, 78.0,
                                        op0=AOP.max, op1=AOP.min)

                # slot equality masks (d in {-1,0,1} wherever weight != 0)
                def eqmasks(base, gbase, tag):
                    dd = wm(tag + "d")
                    nc.vector.tensor_tensor(dd[:], base[:], gbase[:], op=AOP.subtract)
                    es = []
                    for s, v in (("0", 0.0), ("1", 1.0), ("m1", -1.0)):
                        e = wm(tag + "e" + s)
                        nc.vector.tensor_scalar(e[:], dd[:], v, None, op0=AOP.is_equal)
                        es.append(e)
                    return es  # [e0, e1, em1]

                ex0, ex1, exm1 = eqmasks(x0, gx, "x")
                ey0, ey1, eym1 = eqmasks(y0, gy, "y")

                # corner weights with validity
                omty = wm("omty"); omtx = wm("omtx")
                nc.vector.tensor_scalar(omty[:], ty[:], -1.0, 1.0, op0=AOP.mult, op1=AOP.add)
                nc.vector.tensor_scalar(omtx[:], tx[:], -1.0, 1.0, op0=AOP.mult, op1=AOP.add)
                wyv0 = wm("wyv0"); wyv1 = wm("wyv1")
                nc.vector.tensor_tensor(wyv0[:], omty[:], vy0[:], op=AOP.mult)
                nc.vector.tensor_tensor(wyv1[:], ty[:], vy1[:], op=AOP.mult)
                wxv0 = wm("wxv0"); wxv1 = wm("wxv1")
                nc.vector.tensor_tensor(wxv0[:], omtx[:], vx0[:], op=AOP.mult)
                nc.vector.tensor_tensor(wxv1[:], tx[:], vx1[:], op=AOP.mult)

                # slot weights: slot j covers row/col gbase+j
                def slotw(w0v, w1v, e0, e1, em1, tag):
                    t1 = wm(tag + "t1"); t2 = wm(tag + "t2")
                    s0 = wm(tag + "s0"); s1 = wm(tag + "s1")
                    nc.vector.tensor_tensor(t1[:], w0v[:], e0[:], op=AOP.mult)
                    nc.vector.tensor_tensor(t2[:], w1v[:], em1[:], op=AOP.mult)
                    nc.vector.tensor_tensor(s0[:], t1[:], t2[:], op=AOP.add)
                    nc.vector.tensor_tensor(t1[:], w0v[:], e1[:], op=AOP.mult)
                    nc.vector.tensor_tensor(t2[:], w1v[:], e0[:], op=AOP.mult)
                    nc.vector.tensor_tensor(s1[:], t1[:], t2[:], op=AOP.add)
                    return s0, s1

                wsx0, wsx1 = slotw(wxv0, wxv1, ex0, ex1, exm1, "sx")
                wsy0, wsy1 = slotw(wyv0, wyv1, ey0, ey1, eym1, "sy")

                # combined weights [128, kb, t], kb = k*4 + b,
                # elem block b: 0=(y0,x0) 1=(y1,x0) 2=(y0,x1) 3=(y1,x1)
                wgt_t = cpool.tile([128, 36, NT], F32, tag="wgt", name="wgt")
                for bslot, (wyv, wxv) in enumerate(
                        ((wsy0, wsx0), (wsy1, wsx0), (wsy0, wsx1), (wsy1, wsx1))):
                    nc.vector.tensor_tensor(
                        wgt_t[:, bslot:36:4, :].rearrange("p k t -> p t k"),
                        wyv[:], wxv[:], op=AOP.mult)

                # indices: idx = gy*64 + gx - ((16+w0)*64 + 16)  (wconst col 0)
                gxs = wm("gxs")
                nc.vector.tensor_scalar_add(gxs[:], gx[:], wconst[:, 0:1])
                idxf = wmpool.tile([128, NT, K], F32, tag="idxf", name="idxf")
                nc.vector.scalar_tensor_tensor(
                    idxf[:], gy[:], 64.0, gxs[:], op0=AOP.mult, op1=AOP.add)

                # ---- z matmuls for taps 0,1 early so gathers start ASAP -----
                z_tap_group(0, 2)

                # ---- partition shuffle into gather layout, on-chip ----------
                # need idx16s[16a+v, k, t, u] = idxf[16u + v, t, k]
                idx16s = cpool.tile([128, K, NT, 8], I16, tag="idx16s",
                                    name="idx16s")
                for u in range(8):
                    sps = ps2pool.tile([128, NT, K], F32, tag="shps", name="shps")
                    nc.tensor.matmul(
                        sps[:].rearrange("p a b -> p (a b)"),
                        pmat[:, u, :], idxf[:].rearrange("p a b -> p (a b)"),
                        start=True, stop=True)
                    nc.vector.tensor_copy(
                        idx16s[:, :, :, u],
                        sps[:].rearrange("p t k -> p k t"))

                # ---------------- remaining z matmuls ----------------
                for kp in range(2, K, 2):
                    z_tap_group(kp, min(2, K - kp))

                # ---------------- gather + weighted accumulate ----------------
                # GN partial stats are interleaved into the last tap so the
                # tail after the final stt is just the cross-tile reduce + CC.
                acc = accpool.tile([128, NT, COUT], F16, tag="acc", name="acc")
                nc.vector.memset(acc[:], 0)

                psums = wmpool.tile([128, NT, 32], F32, tag="psums", name="psums")
                psqs = wmpool.tile([128, NT, 32], F32, tag="psqs", name="psqs")
                sqt = wmpool.tile([128, COUT], F32, tag="sqt", name="sqt")
                AX = mybir.AxisListType.X
                for k in range(K):
                    zb = zds[k][:]
                    # overlapped view: idx q -> 1024 elems starting at q*512
                    in_ap = bass.AP(zb.tensor, zb.offset,
                                    [[512, ZDR - 1], [1, 1024]])
                    gts = []
                    for hh in range(2):
                        g = gpool.tile([128, 8, 1024], F16,
                                       tag=f"g{hh}", name=f"g{hh}")
                        nc.gpsimd.dma_gather(
                            out_ap=g[:],
                            in_ap=in_ap,
                            idxs_ap=idx16s[:, k, 8 * hh:8 * (hh + 1), :]
                            .rearrange("p a b -> p (a b)"),
                            num_idxs=NPOS // 2,
                            num_idxs_reg=NPOS // 2,
                            elem_size=1024,
                            elem_step=512,
                        )
                        gts.append(g)
                    for t in range(NT):
                        g = gts[t // 8]
                        for bslot in range(4):
                            nc.vector.scalar_tensor_tensor(
                                acc[:, t, :],
                                g[:, t % 8, 256 * bslot:256 * (bslot + 1)],
                                wgt_t[:, 4 * k + bslot, t:t + 1],
                                acc[:, t, :],
                                op0=AOP.mult, op1=AOP.add)
                        if k == K - 1:
                            nc.vector.tensor_reduce(
                                psums[:, t, :],
                                acc[:, t, :].rearrange("p (g c) -> p g c", c=8),
                                axis=AX, op=AOP.add)
                            nc.vector.tensor_tensor(sqt[:], acc[:, t, :],
                                                    acc[:, t, :], op=AOP.mult)
                            nc.vector.tensor_reduce(
                                psqs[:, t, :],
                                sqt[:].rearrange("p (g c) -> p g c", c=8),
                                axis=AX, op=AOP.add)

                # ---------------- GroupNorm stats + AllReduce ----------------
                stats = wmpool.tile([128, 64], F32, tag="stats", name="stats")
                nc.vector.tensor_reduce(
                    stats[:, 0:32], psums[:].rearrange("p t g -> p g t"),
                    axis=AX, op=AOP.add)
                nc.vector.tensor_reduce(
                    stats[:, 32:64], psqs[:].rearrange("p t g -> p g t"),
                    axis=AX, op=AOP.add)
                # partition reduce via ones matmul -> [1, 64]
                sps = ps2pool.tile([1, 64], F32, tag="sps", name="sps")
                nc.tensor.matmul(sps[:], onescol[:], stats[:], start=True, stop=True)
                stat_row = wmpool.tile([1, 64], F32, tag="strow", name="strow")
                nc.vector.tensor_copy(stat_row[:], sps[:])
                nc.sync.dma_start(ccin[:], stat_row[:])
                if use_cc:
                    nc.gpsimd.collective_compute(
                        "AllReduce", AOP.add,
                        replica_groups=[[0, 1], [2, 3], [4, 5], [6, 7]],
                        ins=[ccin[:].opt()], outs=[ccout[:].opt()],
                    )
                else:
                    nc.sync.dma_start(ccout[:], ccin[:])
                allst = wmpool.tile([1, 64], F32, tag="allst", name="allst")
                nc.sync.dma_start(allst[:], ccout[:])

                # mu = S/n; var = Q/n - mu^2; A = gamma*rstd; B = beta - mu*A
                mu = wmpool.tile([1, 32], F32, tag="mu", name="mu")
                var = wmpool.tile([1, 32], F32, tag="var", name="var")
                rstd = wmpool.tile([1, 32], F32, tag="rstd", name="rstd")
                nc.vector.tensor_scalar_mul(mu[:], allst[:, 0:32], 1.0 / GN_N)
                nc.vector.tensor_scalar_mul(var[:], allst[:, 32:64], 1.0 / GN_N)
                nc.vector.tensor_tensor(rstd[:], mu[:], mu[:], op=AOP.mult)
                nc.vector.tensor_tensor(var[:], var[:], rstd[:], op=AOP.subtract)
                nc.vector.tensor_scalar_add(var[:], var[:], EPS)
                nc.scalar.activation(rstd[:], var[:], ACT.Sqrt, bias=0.0)
                nc.vector.reciprocal(rstd[:], rstd[:])
                abrow = wmpool.tile([1, 512], F32, tag="abrow", name="abrow")
                rrep = wmpool.tile([1, 512], F32, tag="rrep", name="rrep")
                # repeat rstd / mu 8x along channels via strided copies
                for c in range(8):
                    nc.vector.tensor_copy(rrep[0:1, c:256:8], rstd[:])
                    nc.vector.tensor_copy(rrep[0:1, 256 + c:512:8], mu[:])
                nc.vector.tensor_tensor(
                    abrow[:, 0:256], rrep[:, 0:256], gnab[:, 0:256], op=AOP.mult)
                nc.vector.tensor_tensor(
                    abrow[:, 256:512], rrep[:, 256:512], abrow[:, 0:256], op=AOP.mult)
                nc.vector.tensor_tensor(
                    abrow[:, 256:512], gnab[:, 256:512], abrow[:, 256:512],
                    op=AOP.subtract)
                # broadcast to [128, 512] via ones-row matmul
                abps = ps2pool.tile([128, 512], F32, tag="abps", name="abps")
                nc.tensor.matmul(abps[:], onesrow[:], abrow[:], start=True, stop=True)
                abbc = cpool.tile([128, 512], F32, tag="abbc", name="abbc")
                nc.scalar.copy(abbc[:], abps[:])

                # ---------------- apply GN + ReLU, write out ----------------
                for t in range(NT):
                    ot = outpool.tile([128, COUT], F32, tag="ot", name="ot")
                    nc.vector.tensor_tensor(ot[:], acc[:, t, :], abbc[:, 0:256], op=AOP.mult)
                    nc.vector.tensor_tensor(ot[:], ot[:], abbc[:, 256:512], op=AOP.add)
                    nc.scalar.activation(ot[:], ot[:], ACT.Relu)
                    od_ap = out_d[:, :]
                    wr = bass.AP(od_ap.tensor, od_ap.offset + t * 128 * COUT,
                                 [[COUT, 128], [1, COUT]])
                    [nc.sync, nc.scalar][t % 2].dma_start(wr, ot[:])

    nc.compile()
    return nc


@functools.lru_cache(maxsize=1)
def _program():
    return build_program()


def _prep_core(core, x, offw, offb, dw):
    b, h = core // 2, core % 2
    r0 = 32 * h
    w0 = r0 - 4

    xsl = np.zeros((2, 128, XROWS, XCOLS), np.float32)
    for i, r in enumerate(range(r0 - 1, r0 + XROWS - 1)):
        if 0 <= r < H:
            xsl[0, :, i, 1:65] = x[b, 0:128, r, :]
            xsl[1, :, i, 1:65] = x[b, 128:256, r, :]
    xzarr = np.zeros((2, 128, WROWS, 64), np.float32)
    for i, r in enumerate(range(w0, w0 + WROWS)):
        if 0 <= r < H:
            xzarr[0, :, i, :] = x[b, 0:128, r, :]
            xzarr[1, :, i, :] = x[b, 128:256, r, :]

    # weights: wdef[ci, c, k, o] = dw[o, ci*128+c, ky, kx]
    dwr = dw.reshape(COUT, CIN, K).transpose(1, 2, 0)     # [cin, k, o]
    wdef = np.ascontiguousarray(
        dwr.reshape(2, 128, K, COUT)).astype(np.float16)
    owr = offw.reshape(18, CIN, K).transpose(1, 2, 0)      # [cin, k, 18]
    woff = np.ascontiguousarray(
        owr.reshape(2, 128, K, 18)).astype(np.float16)

    pos = np.arange(NPOS)
    prow = r0 + pos // 64
    pcol = pos % 64
    ky = np.arange(K) // 3
    kx = np.arange(K) % 3
    # lifted (+16) base grids with offset bias folded in
    by = prow[:, None] - 1.0 + ky[None, :] + offb[0::2][None, :] + 16.0
    bx = pcol[:, None] - 1.0 + kx[None, :] + offb[1::2][None, :] + 16.0
    # [NPOS, K] -> [128, NT, K] with position q at (q%128, q//128)
    byc = by.reshape(NT, 128, K).transpose(1, 0, 2).astype(np.float32)
    bxc = bx.reshape(NT, 128, K).transpose(1, 0, 2).astype(np.float32)

    wconst = np.zeros((128, 4), np.float32)
    wconst[:, 0] = -((16 + w0) * 64 + 16)
    wconst[:, 1] = w0 + 16                # y pair clamp lo (lifted)
    wconst[:, 2] = w0 + 16 + WROWS - 2    # y pair clamp hi (lifted, whi-1)

    return {
        "xsl": np.ascontiguousarray(xsl.astype(np.float16)),
        "xz": np.ascontiguousarray(xzarr.reshape(2, 128, NWIN).astype(np.float16)),
        "wdef": wdef, "woff": woff,
        "byc": np.ascontiguousarray(byc), "bxc": np.ascontiguousarray(bxc),
        "wconst": wconst,
    }


def kernel(x, offset_w, offset_b, deform_w, gn_gamma, gn_beta):
    x = np.asarray(x, np.float32)
    offw = np.asarray(offset_w, np.float32)
    offb = np.asarray(offset_b, np.float32)
    dw = np.asarray(deform_w, np.float32)
    gamma = np.asarray(gn_gamma, np.float32)
    beta = np.asarray(gn_beta, np.float32)

    nc = _program()

    ident = np.eye(128, dtype=np.float32)
    onescol = np.ones((128, 1), np.float32)
    onesrow = np.ones((1, 128), np.float32)
    gnab = np.concatenate([gamma, beta]).reshape(1, 512).astype(np.float32)
    # pmat[s, u, p] = 1 iff s == 16u + p%16
    sig = np.arange(128)[:, None, None]
    uu = np.arange(8)[None, :, None]
    pp = np.arange(128)[None, None, :]
    pmat = (sig == 16 * uu + pp % 16).astype(np.float32)

    in_maps = []
    for core in range(8):
        m = _prep_core(core, x, offw, offb, dw)
        m.update({"ident": ident, "onescol": onescol, "onesrow": onesrow,
                  "gnab": gnab, "pmat": np.ascontiguousarray(pmat)})
        in_maps.append(m)

    global _last_in_maps
    _last_in_maps = in_maps

    res = run_bass_kernel_spmd(nc, in_maps, core_ids=list(range(8)))

    out = np.zeros((B, COUT, H, W), np.float32)
    for core in range(8):
        b, h = core // 2, core % 2
        o = res.results[core]["out"]  # [2048, 256]
        out[b, :, 32 * h:32 * h + 32, :] = (
            o.reshape(32, 64, COUT).transpose(2, 0, 1))
    return out


# revision 15
# speedup vs baseline: 1.1521x; 1.1521x over previous
"""Trainium2 Bass kernel for nn_DeformableBlock (offset-conv -> deformable
conv v1 -> GroupNorm(32) -> ReLU), 8-core SPMD.

Sharding: core c -> (batch b = c//2, row-half h = c%2), rows [32h, 32h+32).
GroupNorm statistics are AllReduce'd across each (b,0)/(b,1) core pair.

Design:
  - z_k = x . W_k per 3x3 tap over a 40-row window, fp16 matmuls (fp32 psum),
    two taps per matmul (512-col moving operand).
  - z stored to DRAM in a doubled-row layout zd[q] = [z[q], z[q+64]] (fp16),
    so ONE gathered element (elem_size=1024, elem_step=512) fetches all 4
    bilinear corners: (y0,x0),(y1,x0),(y0,x1),(y1,x1). 18 dma_gathers total
    (9 taps x 2 halves of 1024 idx) -- SWDGE descriptor gen is the scarce
    resource. Slot weights are equality-adjusted for the y/x edge clamps.
  - gather index layout ([16-partition wrap, replicated x8]) built ON-CHIP via
    8 selection matmuls (partition shuffle), no DRAM bounce.
  - z stores round-robin across sync/scalar engine DMA queues.
"""
import functools
import numpy as np
import ml_dtypes

import concourse.bass as bass
import concourse.bacc as bacc
import concourse.mybir as mybir
import concourse.tile as tile
from concourse.bass_utils import run_bass_kernel_spmd

F32 = mybir.dt.float32
BF16 = mybir.dt.bfloat16
F16 = mybir.dt.float16
I16 = mybir.dt.int16
I32 = mybir.dt.int32
AOP = mybir.AluOpType
ACT = mybir.ActivationFunctionType

B, CIN, COUT, H, W = 4, 256, 256, 64, 64
K = 9
WROWS = 40            # z window rows (rows r0-4 .. r0+35)
XROWS = 35            # padded x slice rows (offset conv only; +1 slack row)
XCOLS = 66
NPOS = 2048           # output positions per core (32 rows)
NWIN = WROWS * 64     # z window positions (2560)
NT = 16               # output position tiles of 128
WT = 20               # window position tiles of 128
HT = WT // 2          # tiles per half-window store
ZDR = NWIN + 2        # zd rows (incl pad)
EPS = 1e-5
GN_N = 2 * NPOS * 8   # elements per GN group (both cores of the pair)

bf16 = ml_dtypes.bfloat16


def build_program(reps=1, use_cc=True):
    nc = bacc.Bacc(None, target_bir_lowering=False, num_devices=8)

    # ---------------- I/O ----------------
    xsl_d = nc.dram_tensor("xsl", [2, 128, XROWS, XCOLS], F16, kind="ExternalInput")
    xz_d = nc.dram_tensor("xz", [2, 128, NWIN], F16, kind="ExternalInput")
    wdef_d = nc.dram_tensor("wdef", [2, 128, K, COUT], F16, kind="ExternalInput")
    woff_d = nc.dram_tensor("woff", [2, 128, K, 18], F16, kind="ExternalInput")
    byc_d = nc.dram_tensor("byc", [128, NT, K], F32, kind="ExternalInput")
    bxc_d = nc.dram_tensor("bxc", [128, NT, K], F32, kind="ExternalInput")
    # per-core scalars replicated to [128,*]: idx offset, window y clamp lo/hi
    wconst_d = nc.dram_tensor("wconst", [128, 4], F32, kind="ExternalInput")
    # partition-shuffle matrices: pmat[s, u, p] = 1 iff s == 16u + p%16
    pmat_d = nc.dram_tensor("pmat", [128, 8, 128], F32, kind="ExternalInput")
    ident_d = nc.dram_tensor("ident", [128, 128], F32, kind="ExternalInput")
    onescol_d = nc.dram_tensor("onescol", [128, 1], F32, kind="ExternalInput")
    onesrow_d = nc.dram_tensor("onesrow", [1, 128], F32, kind="ExternalInput")
    gnab_d = nc.dram_tensor("gnab", [1, 512], F32, kind="ExternalInput")
    out_d = nc.dram_tensor("out", [NPOS, COUT], F32, kind="ExternalOutput")

    with tile.TileContext(nc) as tc:
        with (
            tc.tile_pool(name="const", bufs=1) as cpool,
            tc.tile_pool(name="wm", bufs=1) as wmpool,
            tc.tile_pool(name="zst", bufs=3) as zstpool,
            tc.tile_pool(name="g", bufs=2) as gpool,
            tc.tile_pool(name="acc", bufs=1) as accpool,
            tc.tile_pool(name="outp", bufs=2) as outpool,
            tc.tile_pool(name="ps", bufs=3, space="PSUM") as pspool,
            tc.tile_pool(name="ps2", bufs=1, space="PSUM") as ps2pool,
            tc.tile_pool(name="dram", bufs=1, space="DRAM") as dpool,
        ):
            # ---------------- load constants / inputs ----------------
            xsl = cpool.tile([128, 2, XROWS, XCOLS], F16, tag="xsl", name="xsl")
            for ci in range(2):
                nc.sync.dma_start(xsl[:, ci], xsl_d[ci])
            xz = cpool.tile([128, 2, NWIN], F16, tag="xz", name="xz")
            for ci in range(2):
                nc.sync.dma_start(xz[:, ci], xz_d[ci])
            wdef = cpool.tile([128, 2, K, COUT], F16, tag="wdef", name="wdef")
            woff = cpool.tile([128, 2, K, 18], F16, tag="woff", name="woff")
            for ci in range(2):
                nc.sync.dma_start(wdef[:, ci], wdef_d[ci])
                nc.sync.dma_start(woff[:, ci], woff_d[ci])
            byc = cpool.tile([128, NT, K], F32, tag="byc", name="byc")
            bxc = cpool.tile([128, NT, K], F32, tag="bxc", name="bxc")
            nc.sync.dma_start(byc[:], byc_d[:])
            nc.sync.dma_start(bxc[:], bxc_d[:])
            wconst = cpool.tile([128, 4], F32, tag="wconst", name="wconst")
            nc.sync.dma_start(wconst[:], wconst_d[:])
            pmat = cpool.tile([128, 8, 128], F32, tag="pmat", name="pmat")
            nc.sync.dma_start(pmat[:], pmat_d[:])
            ident = cpool.tile([128, 128], F32, tag="ident", name="ident")
            nc.sync.dma_start(ident[:], ident_d[:])
            onescol = cpool.tile([128, 1], F32, tag="onescol", name="onescol")
            nc.sync.dma_start(onescol[:], onescol_d[:])
            onesrow = cpool.tile([1, 128], F32, tag="onesrow", name="onesrow")
            nc.sync.dma_start(onesrow[:], onesrow_d[:])
            gnab = cpool.tile([1, 512], F32, tag="gnab", name="gnab")
            nc.sync.dma_start(gnab[:], gnab_d[:])

            # one doubled-row dram tile per tap: zd[q] = [z[q], z[q+64]]
            zds = [dpool.tile([ZDR, 2 * COUT], F16, tag=f"zd{k}",
                              name=f"zd{k}") for k in range(K)]
            ccin = dpool.tile([1, 64], F32, tag="ccin", name="ccin")
            ccout = dpool.tile([1, 64], F32, tag="ccout", name="ccout")

            # z matmuls for a pair of taps (or single for the last), plus the
            # doubled-layout stores. eng alternates the issuing DMA queue.
            def z_tap_group(kp, npair):
                fw = 512 if npair == 2 else 256
                for hw in range(2):
                    zst = zstpool.tile([128, HT, 512], F16, tag="zst", name="zst")
                    for tt in range(HT):
                        t = HT * hw + tt
                        zps = pspool.tile([128, 512], F32, tag="zps", name="zps")
                        nc.tensor.matmul(
                            zps[:, 0:fw], xz[:, 0, 128 * t:128 * (t + 1)],
                            wdef[:, 0, kp:kp + npair, :]
                            .rearrange("p a b -> p (a b)"),
                            start=True, stop=False)
                        nc.tensor.matmul(
                            zps[:, 0:fw], xz[:, 1, 128 * t:128 * (t + 1)],
                            wdef[:, 1, kp:kp + npair, :]
                            .rearrange("p a b -> p (a b)"),
                            start=False, stop=True)
                        nc.scalar.copy(zst[:, tt, 0:fw], zps[:, 0:fw])
                    for j in range(npair):
                        k = kp + j
                        eng = [nc.sync, nc.scalar][(k + hw) % 2]
                        zb = zds[k][:]
                        src = zst[:, :, 256 * j:256 * (j + 1)]
                        # write1: zd[q][0:256] = z[q],  q = hw*1280 + 128t + p
                        wr = bass.AP(zb.tensor, zb.offset + hw * 1280 * 512,
                                     [[512, 128], [128 * 512, HT], [1, 256]])
                        eng.dma_start(wr, src)
                        # write2: zd[q-64][256:512] = z[q]
                        if hw == 0:
                            wr = bass.AP(zb.tensor, zb.offset + 256,
                                         [[512, 64], [1, 256]])
                            eng.dma_start(wr, zst[64:128, 0, 256 * j:256 * (j + 1)])
                            wr = bass.AP(zb.tensor, zb.offset + 64 * 512 + 256,
                                         [[512, 128], [128 * 512, HT - 1], [1, 256]])
                            eng.dma_start(wr, zst[:, 1:HT, 256 * j:256 * (j + 1)])
                        else:
                            wr = bass.AP(zb.tensor,
                                         zb.offset + (1280 - 64) * 512 + 256,
                                         [[512, 128], [128 * 512, HT], [1, 256]])
                            eng.dma_start(wr, src)

            for _rep in range(reps):
                # ---------------- offset conv: [18, 2048] via im2col ----------
                # moving operand streams full padded rows (66 cols incl junk),
                # junk skipped at evacuation
                off_sb = cpool.tile([18, NPOS], F32, tag="off_sb", name="off_sb")
                xsl_flat = xsl[:].rearrange("p c r x -> p c (r x)")
                for q in range(6):  # 6-row chunks of output rows (last is 2)
                    nrows = 6 if q < 5 else 2
                    span = nrows * XCOLS
                    ops = ps2pool.tile([18, 6 * XCOLS], F32, tag="offps", name="offps")
                    first = True
                    for k in range(K):
                        ky, kx = k // 3, k % 3
                        base = (6 * q + ky) * XCOLS + kx
                        nc.tensor.matmul(
                            ops[:, 0:span], woff[:, 0, k, :],
                            xsl_flat[:, 0, base:base + span],
                            start=first, stop=False)
                        first = False
                        nc.tensor.matmul(
                            ops[:, 0:span], woff[:, 1, k, :],
                            xsl_flat[:, 1, base:base + span],
                            start=False, stop=(k == K - 1))
                    nc.scalar.copy(
                        off_sb[:, 384 * q:384 * q + 64 * nrows]
                        .rearrange("p (r x) -> p r x", x=64),
                        ops[:, 0:span].rearrange("p (r x) -> p r x", x=XCOLS)[:, :, 0:64])

                # PE-transpose offsets to position-major [128, NT, 18]
                offt = cpool.tile([128, NT, 18], F32, tag="offt", name="offt")
                for t in range(NT):
                    tps = ps2pool.tile([128, 18], F32, tag="tps", name="tps")
                    nc.tensor.transpose(
                        tps[:], off_sb[:, 128 * t:128 * (t + 1)], ident[0:18, 0:18])
                    nc.vector.tensor_copy(offt[:, t, :], tps[:])

                # ---------------- bilinear weights + indices (fp32, DVE) ------
                def wm(tag):
                    return wmpool.tile([128, NT, K], F32, tag=tag, name=tag)

                py = wm("py"); px = wm("px")
                # lifted sample coords: byc/bxc carry +16 and the offset bias
                nc.vector.tensor_add(py[:], offt[:, :, 0:18:2], byc[:])
                nc.vector.tensor_add(px[:], offt[:, :, 1:18:2], bxc[:])

                def dev_floor(src, tag):
                    ii = wmpool.tile([128, NT, K], I32, tag=tag + "i", name=tag + "i")
                    ff = wm(tag + "f")
                    gt = wm(tag + "g")
                    nc.vector.tensor_copy(ii[:], src[:])        # fp32 -> int32
                    nc.vector.tensor_copy(ff[:], ii[:])         # int32 -> fp32
                    nc.vector.tensor_tensor(gt[:], ff[:], src[:], op=AOP.is_gt)
                    nc.vector.tensor_tensor(ff[:], ff[:], gt[:], op=AOP.subtract)
                    return ff

                y0 = dev_floor(py, "y0")
                x0 = dev_floor(px, "x0")
                ty = wm("ty"); tx = wm("tx")
                nc.vector.tensor_tensor(ty[:], py[:], y0[:], op=AOP.subtract)
                nc.vector.tensor_tensor(tx[:], px[:], x0[:], op=AOP.subtract)
                y1 = wm("y1"); x1 = wm("x1")
                nc.vector.tensor_scalar_add(y1[:], y0[:], 1.0)
                nc.vector.tensor_scalar_add(x1[:], x0[:], 1.0)

                # global validity (lifted bounds [16, 79])
                def valid(src, tag):
                    g = wm(tag + "c")
                    v = wm(tag + "v")
                    nc.vector.tensor_scalar(g[:], src[:], 16.0, 79.0,
                                            op0=AOP.max, op1=AOP.min)
                    nc.vector.tensor_tensor(v[:], g[:], src[:], op=AOP.is_equal)
                    return v

                vy0 = valid(y0, "vy0"); vy1 = valid(y1, "vy1")
                vx0 = valid(x0, "vx0"); vx1 = valid(x1, "vx1")

                # gather pair bases: y row to [wlo, whi-1], x col to [16, 78]
                gy = wm("gy"); gx = wm("gx")
                nc.vector.tensor_scalar(gy[:], y0[:], wconst[:, 1:2],
                                        wconst[:, 2:3], op0=AOP.max, op1=AOP.min)
                nc.vector.tensor_scalar(gx[:], x0[:], 16.0, 78.0,
                                        op0=AOP.max, op1=AOP.min)

                # slot equality masks (d in {-1,0,1} wherever weight != 0)
                def eqmasks(base, gbase, tag):
                    dd = wm(tag + "d")
                    nc.vector.tensor_tensor(dd[:], base[:], gbase[:], op=AOP.subtract)
                    es = []
                    for s, v in (("0", 0.0), ("1", 1.0), ("m1", -1.0)):
                        e = wm(tag + "e" + s)
                        nc.vector.tensor_scalar(e[:], dd[:], v, None, op0=AOP.is_equal)
                        es.append(e)
                    return es  # [e0, e1, em1]

                ex0, ex1, exm1 = eqmasks(x0, gx, "x")
                ey0, ey1, eym1 = eqmasks(y0, gy, "y")

                # corner weights with validity
                omty = wm("omty"); omtx = wm("omtx")
                nc.vector.tensor_scalar(omty[:], ty[:], -1.0, 1.0, op0=AOP.mult, op1=AOP.add)
                nc.vector.tensor_scalar(omtx[:], tx[:], -1.0, 1.0, op0=AOP.mult, op1=AOP.add)
                wyv0 = wm("wyv0"); wyv1 = wm("wyv1")
                nc.vector.tensor_tensor(wyv0[:], omty[:], vy0[:], op=AOP.mult)
                nc.vector.tensor_tensor(wyv1[:], ty[:], vy1[:], op=AOP.mult)
                wxv0 = wm("wxv0"); wxv1 = wm("wxv1")
                nc.vector.tensor_tensor(wxv0[:], omtx[:], vx0[:], op=AOP.mult)
                nc.vector.tensor_tensor(wxv1[:], tx[:], vx1[:], op=AOP.mult)

                # slot weights: slot j covers row/col gbase+j
                def slotw(w0v, w1v, e0, e1, em1, tag):
                    t1 = wm(tag + "t1"); t2 = wm(tag + "t2")
                    s0 = wm(tag + "s0"); s1 = wm(tag + "s1")
                    nc.vector.tensor_tensor(t1[:], w0v[:], e0[:], op=AOP.mult)
                    nc.vector.tensor_tensor(t2[:], w1v[:], em1[:], op=AOP.mult)
                    nc.vector.tensor_tensor(s0[:], t1[:], t2[:], op=AOP.add)
                    nc.vector.tensor_tensor(t1[:], w0v[:], e1[:], op=AOP.mult)
                    nc.vector.tensor_tensor(t2[:], w1v[:], e0[:], op=AOP.mult)
                    nc.vector.tensor_tensor(s1[:], t1[:], t2[:], op=AOP.add)
                    return s0, s1

                wsx0, wsx1 = slotw(wxv0, wxv1, ex0, ex1, exm1, "sx")
                wsy0, wsy1 = slotw(wyv0, wyv1, ey0, ey1, eym1, "sy")

                # combined weights [128, kb, t], kb = k*4 + b,
                # elem block b: 0=(y0,x0) 1=(y1,x0) 2=(y0,x1) 3=(y1,x1)
                wgt_t = cpool.tile([128, 36, NT], F32, tag="wgt", name="wgt")
                for bslot, (wyv, wxv) in enumerate(
                        ((wsy0, wsx0), (wsy1, wsx0), (wsy0, wsx1), (wsy1, wsx1))):
                    nc.vector.tensor_tensor(
                        wgt_t[:, bslot:36:4, :].rearrange("p k t -> p t k"),
                        wyv[:], wxv[:], op=AOP.mult)

                # indices: idx = gy*64 + gx - ((16+w0)*64 + 16)  (wconst col 0)
                gxs = wm("gxs")
                nc.vector.tensor_scalar_add(gxs[:], gx[:], wconst[:, 0:1])
                idxf = wmpool.tile([128, NT, K], F32, tag="idxf", name="idxf")
                nc.vector.scalar_tensor_tensor(
                    idxf[:], gy[:], 64.0, gxs[:], op0=AOP.mult, op1=AOP.add)

                # ---- z matmuls for taps 0,1 early so gathers start ASAP -----
                z_tap_group(0, 2)

                # ---- partition shuffle into gather layout, on-chip ----------
                # need idx16s[16a+v, k, t, u] = idxf[16u + v, t, k]
                idx16s = cpool.tile([128, K, NT, 8], I16, tag="idx16s",
                                    name="idx16s")
                for u in range(8):
                    sps = ps2pool.tile([128, NT, K], F32, tag="shps", name="shps")
                    nc.tensor.matmul(
                        sps[:].rearrange("p a b -> p (a b)"),
                        pmat[:, u, :], idxf[:].rearrange("p a b -> p (a b)"),
                        start=True, stop=True)
                    nc.vector.tensor_copy(
                        idx16s[:, :, :, u],
                        sps[:].rearrange("p t k -> p k t"))

                # ---------------- remaining z matmuls ----------------
                for kp in range(2, K, 2):
                    z_tap_group(kp, min(2, K - kp))

                # ---------------- gather + weighted accumulate ----------------
                # GN partial stats are interleaved into the last tap so the
                # tail after the final stt is just the cross-tile reduce + CC.
                acc = accpool.tile([128, NT, COUT], F16, tag="acc", name="acc")
                nc.vector.memset(acc[:], 0)

                psums = wmpool.tile([128, NT, 32], F32, tag="psums", name="psums")
                psqs = wmpool.tile([128, NT, 32], F32, tag="psqs", name="psqs")
                sqt = wmpool.tile([128, COUT], F32, tag="sqt", name="sqt")
                AX = mybir.AxisListType.X
                for k in range(K):
                    zb = zds[k][:]
                    # overlapped view: idx q -> 1024 elems starting at q*512
                    in_ap = bass.AP(zb.tensor, zb.offset,
                                    [[512, ZDR - 1], [1, 1024]])
                    gts = []
                    for hh in range(2):
                        g = gpool.tile([128, 8, 1024], F16,
                                       tag=f"g{hh}", name=f"g{hh}")
                        nc.gpsimd.dma_gather(
                            out_ap=g[:],
                            in_ap=in_ap,
                            idxs_ap=idx16s[:, k, 8 * hh:8 * (hh + 1), :]
                            .rearrange("p a b -> p (a b)"),
                            num_idxs=NPOS // 2,
                            num_idxs_reg=NPOS // 2,
                            elem_size=1024,
                            elem_step=512,
                        )
                        gts.append(g)
                    for t in range(NT):
                        g = gts[t // 8]
                        for bslot in range(4):
                            nc.vector.scalar_tensor_tensor(
                                acc[:, t, :],
                                g[:, t % 8, 256 * bslot:256 * (bslot + 1)],
                                wgt_t[:, 4 * k + bslot, t:t + 1],
                                acc[:, t, :],
                                op0=AOP.mult, op1=AOP.add)
                        if k == K - 1:
                            nc.vector.tensor_reduce(
                                psums[:, t, :],
                                acc[:, t, :].rearrange("p (g c) -> p g c", c=8),
                                axis=AX, op=AOP.add)
                            nc.vector.tensor_tensor(sqt[:], acc[:, t, :],
                                                    acc[:, t, :], op=AOP.mult)
                            nc.vector.tensor_reduce(
                                psqs[:, t, :],
                                sqt[:].rearrange("p (g c) -> p g c", c=8),
                                axis=AX, op=AOP.add)

                # ---------------- GroupNorm stats + AllReduce ----------------
                stats = wmpool.tile([128, 64], F32, tag="stats", name="stats")
                nc.vector.tensor_reduce(
                    stats[:, 0:32], psums[:].rearrange("p t g -> p g t"),
                    axis=AX, op=AOP.add)
                nc.vector.tensor_reduce(
                    stats[:, 32:64], psqs[:].rearrange("p t g -> p g t"),
                    axis=AX, op=AOP.add)
                # partition reduce via ones matmul -> [1, 64]
                sps = ps2pool.tile([1, 64], F32, tag="sps", name="sps")
                nc.tensor.matmul(sps[:], onescol[:], stats[:], start=True, stop=True)
                stat_row = wmpool.tile([1, 64], F32, tag="strow", name="strow")
                nc.vector.tensor_copy(stat_row[:], sps[:])
                nc.sync.dma_start(ccin[:], stat_row[:])
                if use_cc:
                    nc.gpsimd.collective_compute(
                        "AllReduce", AOP.add,
                        replica_groups=[[0, 1], [2, 3], [4, 5], [6, 7]],
                        ins=[ccin[:].opt()], outs=[ccout[:].opt()],
                    )
                else:
                    nc.sync.dma_start(ccout[:], ccin[:])
                allst = wmpool.tile([1, 64], F32, tag="allst", name="allst")
                nc.sync.dma_start(allst[:], ccout[:])

                # mu = S/n; var = Q/n - mu^2; A = gamma*rstd; B = beta - mu*A
                mu = wmpool.tile([1, 32], F32, tag="mu", name="mu")
                var = wmpool.tile([1, 32], F32, tag="var", name="var")
                rstd = wmpool.tile([1, 32], F32, tag="rstd", name="rstd")
                nc.vector.tensor_scalar_mul(mu[:], allst[:, 0:32], 1.0 / GN_N)
                nc.vector.tensor_scalar_mul(var[:], allst[:, 32:64], 1.0 / GN_N)
                nc.vector.tensor_tensor(rstd[:], mu[:], mu[:], op=AOP.mult)
                nc.vector.tensor_tensor(var[:], var[:], rstd[:], op=AOP.subtract)
                nc.vector.tensor_scalar_add(var[:], var[:], EPS)
                nc.scalar.activation(rstd[:], var[:], ACT.Sqrt, bias=0.0)
                nc.vector.reciprocal(rstd[:], rstd[:])
                abrow = wmpool.tile([1, 512], F32, tag="abrow", name="abrow")
                rrep = wmpool.tile([1, 512], F32, tag="rrep", name="rrep")
                # repeat rstd / mu 8x along channels via strided copies
                for c in range(8):
                    nc.vector.tensor_copy(rrep[0:1, c:256:8], rstd[:])
                    nc.vector.tensor_copy(rrep[0:1, 256 + c:512:8], mu[:])
                nc.vector.tensor_tensor(
                    abrow[:, 0:256], rrep[:, 0:256], gnab[:, 0:256], op=AOP.mult)
                nc.vector.tensor_tensor(
                    abrow[:, 256:512], rrep[:, 256:512], abrow[:, 0:256], op=AOP.mult)
                nc.vector.tensor_tensor(
                    abrow[:, 256:512], gnab[:, 256:512], abrow[:, 256:512],
                    op=AOP.subtract)
                # broadcast to [128, 512] via ones-row matmul
                abps = ps2pool.tile([128, 512], F32, tag="abps", name="abps")
                nc.tensor.matmul(abps[:], onesrow[:], abrow[:], start=True, stop=True)
                abbc = cpool.tile([128, 512], F32, tag="abbc", name="abbc")
                nc.scalar.copy(abbc[:], abps[:])

                # ---------------- apply GN + ReLU, write out ----------------
                for t in range(NT):
                    ot = outpool.tile([128, COUT], F32, tag="ot", name="ot")
                    nc.vector.tensor_tensor(ot[:], acc[:, t, :], abbc[:, 0:256], op=AOP.mult)
                    nc.vector.tensor_tensor(ot[:], ot[:], abbc[:, 256:512], op=AOP.add)
                    nc.scalar.activation(ot[:], ot[:], ACT.Relu)
                    od_ap = out_d[:, :]
                    wr = bass.AP(od_ap.tensor, od_ap.offset + t * 128 * COUT,
                                 [[COUT, 128], [1, COUT]])
                    [nc.sync, nc.scalar][t % 2].dma_start(wr, ot[:])

    nc.compile()
    return nc


@functools.lru_cache(maxsize=1)
def _program():
    return build_program()


def _prep_core(core, x, offw, offb, dw):
    b, h = core // 2, core % 2
    r0 = 32 * h
    w0 = r0 - 4

    xsl = np.zeros((2, 128, XROWS, XCOLS), np.float32)
    for i, r in enumerate(range(r0 - 1, r0 + XROWS - 1)):
        if 0 <= r < H:
            xsl[0, :, i, 1:65] = x[b, 0:128, r, :]
            xsl[1, :, i, 1:65] = x[b, 128:256, r, :]
    xzarr = np.zeros((2, 128, WROWS, 64), np.float32)
    for i, r in enumerate(range(w0, w0 + WROWS)):
        if 0 <= r < H:
            xzarr[0, :, i, :] = x[b, 0:128, r, :]
            xzarr[1, :, i, :] = x[b, 128:256, r, :]

    # weights: wdef[ci, c, k, o] = dw[o, ci*128+c, ky, kx]
    dwr = dw.reshape(COUT, CIN, K).transpose(1, 2, 0)     # [cin, k, o]
    wdef = np.ascontiguousarray(
        dwr.reshape(2, 128, K, COUT)).astype(np.float16)
    owr = offw.reshape(18, CIN, K).transpose(1, 2, 0)      # [cin, k, 18]
    woff = np.ascontiguousarray(
        owr.reshape(2, 128, K, 18)).astype(np.float16)

    pos = np.arange(NPOS)
    prow = r0 + pos // 64
    pcol = pos % 64
    ky = np.arange(K) // 3
    kx = np.arange(K) % 3
    # lifted (+16) base grids with offset bias folded in
    by = prow[:, None] - 1.0 + ky[None, :] + offb[0::2][None, :] + 16.0
    bx = pcol[:, None] - 1.0 + kx[None, :] + offb[1::2][None, :] + 16.0
    # [NPOS, K] -> [128, NT, K] with position q at (q%128, q//128)
    byc = by.reshape(NT, 128, K).transpose(1, 0, 2).astype(np.float32)
    bxc = bx.reshape(NT, 128, K).transpose(1, 0, 2).astype(np.float32)

    wconst = np.zeros((128, 4), np.float32)
    wconst[:, 0] = -((16 + w0) * 64 + 16)
    wconst[:, 1] = w0 + 16                # y pair clamp lo (lifted)
    wconst[:, 2] = w0 + 16 + WROWS - 2    # y pair clamp hi (lifted, whi-1)

    return {
        "xsl": np.ascontiguousarray(xsl.astype(np.float16)),
        "xz": np.ascontiguousarray(xzarr.reshape(2, 128, NWIN).astype(np.float16)),
        "wdef": wdef, "woff": woff,
        "byc": np.ascontiguousarray(byc), "bxc": np.ascontiguousarray(bxc),
        "wconst": wconst,
    }


def kernel(x, offset_w, offset_b, deform_w, gn_gamma, gn_beta):
    x = np.asarray(x, np.float32)
    offw = np.asarray(offset_w, np.float32)
    offb = np.asarray(offset_b, np.float32)
    dw = np.asarray(deform_w, np.float32)
    gamma = np.asarray(gn_gamma, np.float32)
    beta = np.asarray(gn_beta, np.float32)

    nc = _program()

    ident = np.eye(128, dtype=np.float32)
    onescol = np.ones((128, 1), np.float32)
    onesrow = np.ones((1, 128), np.float32)
    gnab = np.concatenate([gamma, beta]).reshape(1, 512).astype(np.float32)
    # pmat[s, u, p] = 1 iff s == 16u + p%16
    sig = np.arange(128)[:, None, None]
    uu = np.arange(8)[None, :, None]
    pp = np.arange(128)[None, None, :]
    pmat = (sig == 16 * uu + pp % 16).astype(np.float32)

    in_maps = []
    for core in range(8):
        m = _prep_core(core, x, offw, offb, dw)
        m.update({"ident": ident, "onescol": onescol, "onesrow": onesrow,
                  "gnab": gnab, "pmat": np.ascontiguousarray(pmat)})
        in_maps.append(m)

    global _last_in_maps
    _last_in_maps = in_maps

    res = run_bass_kernel_spmd(nc, in_maps, core_ids=list(range(8)))

    out = np.zeros((B, COUT, H, W), np.float32)
    for core in range(8):
        b, h = core // 2, core % 2
        o = res.results[core]["out"]  # [2048, 256]
        out[b, :, 32 * h:32 * h + 32, :] = (
            o.reshape(32, 64, COUT).transpose(2, 0, 1))
    return out
